# revision 28
# baseline (speedup 1.0000x reference)
"""ArcFace logits kernel for 8 Trainium2 NeuronCores.

out = (cos + one_hot_margin_body) * S  where cos = l2norm(x) @ l2norm(weight).T

Sharding: model-parallel over the class dim (12500 classes per core, padded to
12544).  x is replicated.

Division of labor:
  host  - l2-normalizes x and weight (f32), folds S into x, transposes and
          casts to bf16, slices the weight shard per core, gathers the <=256
          (x_row, w_target) pairs owned by each core's shard.
  device- the [1024, 12544] bf16 GEMM (all the FLOPs) plus the per-target
          arccos-margin values (256 slots, f32), returned as a tiny side
          output `corr`.
  host  - concatenates the 8 logit slices and writes the 1024 margin values
          into their (row, target) positions during unshard.

The device inner loop is a pure matmul stream: 7 column groups x 8 batch
tiles x 4 psum banks x 4 contraction chunks, PSUM evacuated by cheap
copies alternating between the Vector and Scalar engines, output DMA'd
per (batch tile, group).  No device-side normalization, no scatter, no
cross-engine dependency chains on the critical path.
"""

import math
import sys
import types

sys.path.insert(0, "/opt/trn_rl_repo")

import numpy as np
import ml_dtypes

# ---- register the NTFF profile hook that the container's antenv lacks ------
# (harmless if profiling is never requested; required for trace=True runs)
def _ensure_axon_hooks():
    try:
        import antenv
        if "antenv.axon_hooks" in sys.modules:
            return
        holder = {"h": None}
        mod = types.ModuleType("antenv.axon_hooks")
        mod.set_axon_ntff_profile_hook = lambda h: holder.__setitem__("h", h)
        mod.get_axon_ntff_profile_hook = lambda: holder["h"]
        sys.modules["antenv.axon_hooks"] = mod
        antenv.axon_hooks = mod
        try:
            from trn_agent_boot.trn_boot import _ntff_profile_via_ctypes
            mod.set_axon_ntff_profile_hook(
                _ntff_profile_via_ctypes("/opt/axon/libaxon_pjrt.so")
            )
        except Exception:
            pass
    except Exception:
        pass


_ensure_axon_hooks()

import concourse.bass as bass
import concourse.mybir as mybir
import concourse.tile as tile
from concourse import bacc
from concourse.tile import add_dep_helper
import concourse.bass_utils as bass_utils

bass_utils.upload_artifacts = lambda tmpdir: tmpdir  # no cloud in container

B = 1024
D = 512
C = 100000
NCORES = 8
CS = C // NCORES          # 12500 classes per core
CSP = 12544               # padded to 98 * 128
S = 64.0
ARC_M = 0.5
COS_M = math.cos(ARC_M)
SIN_M = math.sin(ARC_M)
EPS = 1e-12
NSLOT = 256               # margin slots (2 partition tiles of 128)
DT = D // 128             # 4 contraction chunks
BT = B // 128             # 8 batch tiles
JC = 448                  # columns per psum chunk
NJ = 4                    # psum chunks per group
GCOL = JC * NJ            # 1792 columns per group
NG = CSP // GCOL          # 7 groups
assert NG * GCOL == CSP

f32 = mybir.dt.float32
bf16 = mybir.dt.bfloat16

_CACHE = {}


def _build_graph():
    nc = bacc.Bacc("TRN2", target_bir_lowering=False, debug=False,
                   num_devices=NCORES)

    # weights arrive host-packed as [128,1792] tiles, (g,d)-major, so every
    # tile DMA is one contiguous 448KB HBM read (the DGE merges rows into
    # 7168B packets; per-queue DMA throughput is proportional to packet
    # size, so large packets are essential).
    xt_ext = nc.dram_tensor("xt", [D, B], bf16, kind="ExternalInput")
    wtr_ext = nc.dram_tensor("wtr", [NG * DT * 128, GCOL], bf16,
                             kind="ExternalInput")
    xs_ext = nc.dram_tensor("xs", [NSLOT, D], bf16, kind="ExternalInput")
    ws_ext = nc.dram_tensor("ws", [NSLOT, D], bf16, kind="ExternalInput")
    out_ext = nc.dram_tensor("out", [B, CSP], bf16, kind="ExternalOutput")
    corr_ext = nc.dram_tensor("corr", [NSLOT, 1], f32, kind="ExternalOutput")

    with tile.TileContext(nc) as tc:
        with (
            tc.tile_pool(name="const", bufs=1) as constp,
            tc.tile_pool(name="xt", bufs=1) as xtp,
            tc.tile_pool(name="wt", bufs=1) as wtp,
            tc.tile_pool(name="outsb", bufs=4) as obp,
            tc.tile_pool(name="mslot", bufs=1) as mslotp,
            tc.tile_pool(name="mscr", bufs=2) as mscrp,
            tc.tile_pool(name="msmall", bufs=2) as msmallp,
            tc.tile_pool(name="psum", bufs=8, space="PSUM") as psump,
        ):
            ones_b = constp.tile([128, 512], bf16, tag="ones_b")
            nc.vector.memset(ones_b[:], 1.0)

            # ---- prologue DMA issue order is consumption order ------------
            # The three DMA-capable queues (sync/SP, scalar/Act, gpsimd/Pool)
            # execute their transfers FIFO and share the 16 DMA engines, so
            # the critical first bytes (xt d0/d1, group 0 tiles) head the
            # sync and scalar queues while everything else rides gpsimd.
            wT = {}
            for g in range(NG):
                for d in range(DT):
                    wT[(g, d)] = wtp.tile([128, GCOL], bf16,
                                          tag=f"wT{g}_{d}", name=f"wT{g}_{d}")

            # consumption-ordered, load-balanced across the three DMA
            # queues; bt0 of group 0 runs d-outer so compute starts once
            # wT(0,0) lands.
            xnT = [xtp.tile([128, B], bf16, tag=f"xnT{d}", name=f"xnT{d}")
                   for d in range(DT)]
            nc.sync.dma_start(out=xnT[0][:], in_=xt_ext[0:128, :])
            nc.scalar.dma_start(out=xnT[1][:], in_=xt_ext[128:256, :])
            nc.gpsimd.dma_start(out=xnT[2][:], in_=xt_ext[256:384, :])
            nc.gpsimd.dma_start(out=xnT[3][:], in_=xt_ext[384:512, :])
            nc.sync.dma_start(out=wT[(0, 0)][:], in_=wtr_ext[0:128, :])
            nc.scalar.dma_start(out=wT[(0, 1)][:], in_=wtr_ext[128:256, :])
            nc.gpsimd.dma_start(out=wT[(0, 2)][:], in_=wtr_ext[256:384, :])
            nc.sync.dma_start(out=wT[(0, 3)][:], in_=wtr_ext[384:512, :])

            # margin slot and group-1 weight DMAs are gated on the first
            # real matmul, keeping the early DMA rounds for xt + group 0.
            gated_dmas = []
            mslots = []
            for st in range(NSLOT // 128):
                xs_t = mslotp.tile([128, D], bf16, tag=f"xs{st}")
                ws_t = mslotp.tile([128, D], bf16, tag=f"ws{st}")
                gated_dmas.append(nc.gpsimd.dma_start(
                    out=xs_t[:], in_=xs_ext[st * 128:(st + 1) * 128, :]))
                gated_dmas.append(nc.gpsimd.dma_start(
                    out=ws_t[:], in_=ws_ext[st * 128:(st + 1) * 128, :]))
                mslots.append((xs_t, ws_t))

            for d in range(DT):
                k = DT + d
                gated_dmas.append(nc.gpsimd.dma_start(
                    out=wT[(1, d)][:],
                    in_=wtr_ext[k * 128:(k + 1) * 128, :]))

            # ---- PE warm-up: ramp the p-state while group 0 lands ---------
            for wi in range(5):
                pw = psump.tile([128, 512], f32, tag="po")
                nc.tensor.matmul(out=pw[:, :], lhsT=ones_b[:, 0:128],
                                 rhs=ones_b[:], start=True, stop=True)

            # ---- margin math, emitted mid-stream so it never heads the ----
            # vector/scalar queues while evacuations are due.
            # corr = S * (cond ? cos(arccos(cos_t)+M) : cos_t) per slot.
            parts = []

            def _margin_part_a():
                for st in range(NSLOT // 128):
                    xs_t, ws_t = mslots[st]
                    scr = mscrp.tile([128, D], f32, tag="scr")
                    cost = msmallp.tile([128, 1], f32, tag=f"cost{st}")
                    nc.vector.tensor_tensor(out=scr[:], in0=xs_t[:],
                                            in1=ws_t[:],
                                            op=mybir.AluOpType.mult)
                    nc.vector.tensor_reduce(out=cost[:], in_=scr[:],
                                            axis=mybir.AxisListType.X,
                                            op=mybir.AluOpType.add)
                    u = msmallp.tile([128, 1], f32, tag=f"u{st}")
                    nc.vector.tensor_scalar(out=u[:], in0=cost[:],
                                            scalar1=-1.0, scalar2=1.0,
                                            op0=mybir.AluOpType.max,
                                            op1=mybir.AluOpType.min)
                    usq = msmallp.tile([128, 1], f32, tag=f"usq{st}")
                    nc.vector.tensor_tensor(out=usq[:], in0=u[:], in1=u[:],
                                            op=mybir.AluOpType.mult)
                    root = msmallp.tile([128, 1], f32, tag=f"root{st}")
                    nc.scalar.activation(out=root[:], in_=usq[:],
                                         func=mybir.ActivationFunctionType.Sqrt,
                                         scale=-1.0, bias=1.0)
                    parts.append((cost, u, root))

            def _margin_part_b():
                for st in range(NSLOT // 128):
                    cost, u, root = parts[st]
                    t1 = msmallp.tile([128, 1], f32, tag=f"t1{st}")
                    nc.vector.tensor_scalar(out=t1[:], in0=u[:],
                                            scalar1=COS_M, scalar2=None,
                                            op0=mybir.AluOpType.mult)
                    t2 = msmallp.tile([128, 1], f32, tag=f"t2{st}")
                    nc.vector.tensor_scalar(out=t2[:], in0=root[:],
                                            scalar1=SIN_M, scalar2=None,
                                            op0=mybir.AluOpType.mult)
                    newz = msmallp.tile([128, 1], f32, tag=f"newz{st}")
                    nc.vector.tensor_tensor(out=newz[:], in0=t1[:],
                                            in1=t2[:],
                                            op=mybir.AluOpType.subtract)
                    dlt = msmallp.tile([128, 1], f32, tag=f"dlt{st}")
                    nc.vector.tensor_tensor(out=dlt[:], in0=newz[:],
                                            in1=cost[:],
                                            op=mybir.AluOpType.subtract)
                    mask = msmallp.tile([128, 1], f32, tag=f"mask{st}")
                    nc.vector.tensor_scalar(out=mask[:], in0=cost[:],
                                            scalar1=0.0, scalar2=None,
                                            op0=mybir.AluOpType.is_gt)
                    md = msmallp.tile([128, 1], f32, tag=f"md{st}")
                    nc.vector.tensor_tensor(out=md[:], in0=mask[:],
                                            in1=dlt[:],
                                            op=mybir.AluOpType.mult)
                    val = msmallp.tile([128, 1], f32, tag=f"val{st}")
                    nc.vector.tensor_tensor(out=val[:], in0=cost[:],
                                            in1=md[:],
                                            op=mybir.AluOpType.add)
                    corr = msmallp.tile([128, 1], f32, tag=f"corr{st}")
                    nc.vector.tensor_scalar(out=corr[:], in0=val[:],
                                            scalar1=S, scalar2=None,
                                            op0=mybir.AluOpType.mult)
                    nc.gpsimd.dma_start(
                        out=corr_ext[st * 128:(st + 1) * 128, :],
                        in_=corr[:])

            # ---- main matmul stream ---------------------------------------
            first_mm = None
            for g in range(NG):
                for bt in range(BT):
                    ob = obp.tile([128, GCOL], bf16, tag="ob")
                    pos = [psump.tile([128, 512], f32, tag="po",
                                      name=f"po{j}")
                           for j in range(NJ)]
                    if g == 0 and bt == 0:
                        # d-outer: the first matmuls need only wT(0,0), so
                        # compute starts as soon as the first 448KB lands
                        mm_iter = [(j, dd) for dd in range(DT)
                                   for j in range(NJ)]
                    else:
                        mm_iter = [(j, dd) for j in range(NJ)
                                   for dd in range(DT)]
                    for j, dd in mm_iter:
                        mm = nc.tensor.matmul(
                            out=pos[j][:, :JC],
                            lhsT=xnT[dd][:, bt * 128:(bt + 1) * 128],
                            rhs=wT[(g, dd)][:, j * JC:(j + 1) * JC],
                            start=(dd == 0), stop=(dd == DT - 1))
                        if first_mm is None:
                            first_mm = mm
                            for gd in gated_dmas:
                                add_dep_helper(gd.ins, first_mm.ins,
                                               sync=True,
                                               reason="clear prologue dma")
                    first_evac = None
                    for j in range(NJ):
                        osl = slice(j * JC, (j + 1) * JC)
                        if j < 2:
                            ev = nc.vector.tensor_scalar(
                                out=ob[:, osl], in0=pos[j][:, :JC],
                                scalar1=1.0, scalar2=None,
                                op0=mybir.AluOpType.mult)
                            if first_evac is None:
                                first_evac = ev
                        else:
                            nc.scalar.copy(out=ob[:, osl], in_=pos[j][:, :JC])
                    if bt == BT - 1 and g == NG - 1:
                        # split the final store so it starts right after the
                        # first half's evacuations instead of after all four
                        nc.sync.dma_start(
                            out=out_ext[bt * 128:(bt + 1) * 128,
                                        g * GCOL:g * GCOL + 2 * JC],
                            in_=ob[:, :2 * JC])
                        nc.sync.dma_start(
                            out=out_ext[bt * 128:(bt + 1) * 128,
                                        g * GCOL + 2 * JC:(g + 1) * GCOL],
                            in_=ob[:, 2 * JC:])
                    else:
                        nc.sync.dma_start(
                            out=out_ext[bt * 128:(bt + 1) * 128,
                                        g * GCOL:(g + 1) * GCOL],
                            in_=ob[:])
                    if bt == 0 and g + 2 < NG:
                        # stagger the weight prefetch for group g+2: issue
                        # only after group g has started evacuating
                        for d in range(DT):
                            k = (g + 2) * DT + d
                            dma = nc.gpsimd.dma_start(
                                out=wT[(g + 2, d)][:],
                                in_=wtr_ext[k * 128:(k + 1) * 128, :])
                            add_dep_helper(dma.ins, first_evac.ins, sync=True,
                                           reason="stagger wt prefetch")
                    if g == 0 and bt == 4:
                        _margin_part_a()
                    if g == 1 and bt == 1:
                        _margin_part_b()

    nc.finalize()
    return nc


def _get_graph():
    if "nc" not in _CACHE:
        _CACHE["nc"] = _build_graph()
    return _CACHE["nc"]


def _margin_val(cos_t):
    """Reference margin math (f32 scalar), for host fallback slots."""
    u = min(max(float(cos_t), -1.0), 1.0)
    new = COS_M * u - SIN_M * math.sqrt(max(0.0, 1.0 - u * u))
    return S * (new if cos_t > 0 else float(cos_t))


def kernel(x, weight, target):
    x = np.ascontiguousarray(np.asarray(x, dtype=np.float32))
    weight = np.ascontiguousarray(np.asarray(weight, dtype=np.float32))
    target = np.asarray(target).astype(np.int64)

    nc = _get_graph()

    # host-side l2 normalization (f32, matching the reference's eps clamp)
    xn = x / np.maximum(
        np.sqrt(np.einsum("bd,bd->b", x, x)[:, None]), EPS).astype(np.float32)
    wnrm = np.sqrt(np.einsum("cd,cd->c", weight, weight))[:, None]
    wn = weight / np.maximum(wnrm, EPS).astype(np.float32)

    xt = np.ascontiguousarray((S * xn).T).astype(ml_dtypes.bfloat16)

    e1 = np.zeros((D,), dtype=np.float32)
    e1[0] = 1.0

    in_maps = []
    slot_rows = []      # per core: row indices whose corr the device computes
    host_fixups = []    # (b, t, value) computed on host for overflow slots
    for c in range(NCORES):
        c0 = c * CS
        wsh = np.zeros((D, CSP), dtype=ml_dtypes.bfloat16)
        wsh[:, :CS] = wn[c0:c0 + CS].T.astype(ml_dtypes.bfloat16)
        # pack [128,1792] tiles contiguously, (g,d)-major
        wtr = np.empty((NG * DT * 128, GCOL), dtype=ml_dtypes.bfloat16)
        for g in range(NG):
            for d in range(DT):
                k = g * DT + d
                wtr[k * 128:(k + 1) * 128] = \
                    wsh[d * 128:(d + 1) * 128, g * GCOL:(g + 1) * GCOL]

        rows = np.nonzero((target >= c0) & (target < c0 + CS))[0]
        dev_rows = rows[:NSLOT]
        for b in rows[NSLOT:]:
            t = int(target[b])
            cos_t = float(xn[b] @ wn[t])
            host_fixups.append((int(b), t, _margin_val(cos_t)))
        xs = np.broadcast_to(e1, (NSLOT, D)).copy()
        ws = np.broadcast_to(e1, (NSLOT, D)).copy()
        n = len(dev_rows)
        xs[:n] = xn[dev_rows]
        ws[:n] = wn[target[dev_rows]]
        xs = xs.astype(ml_dtypes.bfloat16)
        ws = ws.astype(ml_dtypes.bfloat16)
        slot_rows.append(dev_rows)
        in_maps.append({"xt": xt, "wtr": wtr, "xs": xs, "ws": ws})

    from concourse.bass_utils import run_bass_kernel_spmd
    res = None
    last_err = None
    for attempt in range(3):
        try:
            res = run_bass_kernel_spmd(nc, in_maps, core_ids=list(range(NCORES)))
            break
        except Exception as e:  # transient NRT_EXEC_UNIT_UNRECOVERABLE flakes
            last_err = e
            import time as _time
            _time.sleep(5)
    if res is None:
        raise last_err

    out = np.concatenate(
        [res.results[c]["out"][:, :CS].astype(np.float32) for c in range(NCORES)],
        axis=1)

    # place the device-computed margin values during unshard
    for c in range(NCORES):
        rows = slot_rows[c]
        if len(rows):
            corr = res.results[c]["corr"][:len(rows), 0].astype(np.float32)
            out[rows, target[rows]] = corr
    for b, t, v in host_fixups:
        out[b, t] = v
    return out


# revision 30
# speedup vs baseline: 1.1893x; 1.1893x over previous
"""ArcFace logits kernel for 8 Trainium2 NeuronCores.

out = (cos + one_hot_margin_body) * S  where cos = l2norm(x) @ l2norm(weight).T

Sharding: model-parallel over the class dim (12500 classes per core, padded to
12544).  x is replicated.

Division of labor:
  host  - l2-normalizes x and weight (f32), folds S into x, transposes and
          casts to bf16, slices the weight shard per core, gathers the <=256
          (x_row, w_target) pairs owned by each core's shard.
  device- the [1024, 12544] bf16 GEMM (all the FLOPs) plus the per-target
          arccos-margin values (256 slots, f32), returned as a tiny side
          output `corr`.
  host  - concatenates the 8 logit slices and writes the 1024 margin values
          into their (row, target) positions during unshard.

The device inner loop is a pure matmul stream: 7 column groups x 8 batch
tiles x 4 psum banks x 4 contraction chunks, PSUM evacuated by cheap
copies alternating between the Vector and Scalar engines, output DMA'd
per (batch tile, group).  No device-side normalization, no scatter, no
cross-engine dependency chains on the critical path.
"""

import math
import sys
import types

sys.path.insert(0, "/opt/trn_rl_repo")

import numpy as np
import ml_dtypes

# ---- register the NTFF profile hook that the container's antenv lacks ------
# (harmless if profiling is never requested; required for trace=True runs)
def _ensure_axon_hooks():
    try:
        import antenv
        if "antenv.axon_hooks" in sys.modules:
            return
        holder = {"h": None}
        mod = types.ModuleType("antenv.axon_hooks")
        mod.set_axon_ntff_profile_hook = lambda h: holder.__setitem__("h", h)
        mod.get_axon_ntff_profile_hook = lambda: holder["h"]
        sys.modules["antenv.axon_hooks"] = mod
        antenv.axon_hooks = mod
        try:
            from trn_agent_boot.trn_boot import _ntff_profile_via_ctypes
            mod.set_axon_ntff_profile_hook(
                _ntff_profile_via_ctypes("/opt/axon/libaxon_pjrt.so")
            )
        except Exception:
            pass
    except Exception:
        pass


_ensure_axon_hooks()

import concourse.bass as bass
import concourse.mybir as mybir
import concourse.tile as tile
from concourse import bacc
from concourse.tile import add_dep_helper
import concourse.bass_utils as bass_utils

bass_utils.upload_artifacts = lambda tmpdir: tmpdir  # no cloud in container

B = 1024
D = 512
C = 100000
NCORES = 8
CS = C // NCORES          # 12500 classes per core
CSP = 12544               # padded to 98 * 128
S = 64.0
ARC_M = 0.5
COS_M = math.cos(ARC_M)
SIN_M = math.sin(ARC_M)
EPS = 1e-12
NSLOT = 256               # margin slots (2 partition tiles of 128)
DT = D // 128             # 4 contraction chunks
BT = B // 128             # 8 batch tiles
JC = 448                  # columns per psum chunk
NJ = 4                    # psum chunks per group
GCOL = JC * NJ            # 1792 columns per group
NG = CSP // GCOL          # 7 groups
assert NG * GCOL == CSP

f32 = mybir.dt.float32
bf16 = mybir.dt.bfloat16

_CACHE = {}


def _build_graph():
    nc = bacc.Bacc("TRN2", target_bir_lowering=False, debug=False,
                   num_devices=NCORES)

    # weights arrive host-packed as [128,1792] tiles, (g,d)-major, so every
    # tile DMA is one contiguous 448KB HBM read (the DGE merges rows into
    # 7168B packets; per-queue DMA throughput is proportional to packet
    # size, so large packets are essential).
    xt_ext = nc.dram_tensor("xt", [D, B], bf16, kind="ExternalInput")
    wtr_ext = nc.dram_tensor("wtr", [NG * DT * 128, GCOL], bf16,
                             kind="ExternalInput")
    xs_ext = nc.dram_tensor("xs", [NSLOT, D], bf16, kind="ExternalInput")
    ws_ext = nc.dram_tensor("ws", [NSLOT, D], bf16, kind="ExternalInput")
    out_ext = nc.dram_tensor("out", [B, CSP], bf16, kind="ExternalOutput")
    corr_ext = nc.dram_tensor("corr", [NSLOT, 1], f32, kind="ExternalOutput")

    with tile.TileContext(nc) as tc:
        with (
            tc.tile_pool(name="const", bufs=1) as constp,
            tc.tile_pool(name="xt", bufs=1) as xtp,
            tc.tile_pool(name="wt", bufs=1) as wtp,
            tc.tile_pool(name="outsb", bufs=4) as obp,
            tc.tile_pool(name="mslot", bufs=1) as mslotp,
            tc.tile_pool(name="mscr", bufs=2) as mscrp,
            tc.tile_pool(name="msmall", bufs=2) as msmallp,
            tc.tile_pool(name="psum", bufs=8, space="PSUM") as psump,
        ):
            ones_b = constp.tile([128, 512], bf16, tag="ones_b")
            nc.vector.memset(ones_b[:], 1.0)

            # ---- prologue DMA issue order is consumption order ------------
            # The three DMA-capable queues (sync/SP, scalar/Act, gpsimd/Pool)
            # execute their transfers FIFO and share the 16 DMA engines, so
            # the critical first bytes (xt d0/d1, group 0 tiles) head the
            # sync and scalar queues while everything else rides gpsimd.
            wT = {}
            for g in range(NG):
                for d in range(DT):
                    wT[(g, d)] = wtp.tile([128, GCOL], bf16,
                                          tag=f"wT{g}_{d}", name=f"wT{g}_{d}")

            # consumption-ordered, load-balanced across the three DMA
            # queues; bt0 of group 0 runs d-outer so compute starts once
            # wT(0,0) lands.
            xnT = [xtp.tile([128, B], bf16, tag=f"xnT{d}", name=f"xnT{d}")
                   for d in range(DT)]
            nc.sync.dma_start(out=xnT[0][:], in_=xt_ext[0:128, :])
            nc.scalar.dma_start(out=xnT[1][:], in_=xt_ext[128:256, :])
            nc.gpsimd.dma_start(out=xnT[2][:], in_=xt_ext[256:384, :])
            nc.gpsimd.dma_start(out=xnT[3][:], in_=xt_ext[384:512, :])
            nc.sync.dma_start(out=wT[(0, 0)][:], in_=wtr_ext[0:128, :])
            nc.scalar.dma_start(out=wT[(0, 1)][:], in_=wtr_ext[128:256, :])
            nc.gpsimd.dma_start(out=wT[(0, 2)][:], in_=wtr_ext[256:384, :])
            nc.sync.dma_start(out=wT[(0, 3)][:], in_=wtr_ext[384:512, :])

            # margin slot and group-1 weight DMAs are gated on the first
            # real matmul, keeping the early DMA rounds for xt + group 0.
            gated_dmas = []
            mslots = []
            for st in range(NSLOT // 128):
                xs_t = mslotp.tile([128, D], bf16, tag=f"xs{st}")
                ws_t = mslotp.tile([128, D], bf16, tag=f"ws{st}")
                gated_dmas.append(nc.gpsimd.dma_start(
                    out=xs_t[:], in_=xs_ext[st * 128:(st + 1) * 128, :]))
                gated_dmas.append(nc.gpsimd.dma_start(
                    out=ws_t[:], in_=ws_ext[st * 128:(st + 1) * 128, :]))
                mslots.append((xs_t, ws_t))

            for d in range(DT):
                k = DT + d
                gated_dmas.append(nc.gpsimd.dma_start(
                    out=wT[(1, d)][:],
                    in_=wtr_ext[k * 128:(k + 1) * 128, :]))

            # ---- PE warm-up: ramp the p-state while group 0 lands ---------
            for wi in range(8):
                pw = psump.tile([128, 512], f32, tag="po")
                nc.tensor.matmul(out=pw[:, :], lhsT=ones_b[:, 0:128],
                                 rhs=ones_b[:], start=True, stop=True)

            # ---- margin math, emitted mid-stream so it never heads the ----
            # vector/scalar queues while evacuations are due.
            # corr = S * (cond ? cos(arccos(cos_t)+M) : cos_t) per slot.
            parts = []

            def _margin_part_a():
                for st in range(NSLOT // 128):
                    xs_t, ws_t = mslots[st]
                    scr = mscrp.tile([128, D], f32, tag="scr")
                    cost = msmallp.tile([128, 1], f32, tag=f"cost{st}")
                    nc.vector.tensor_tensor(out=scr[:], in0=xs_t[:],
                                            in1=ws_t[:],
                                            op=mybir.AluOpType.mult)
                    nc.vector.tensor_reduce(out=cost[:], in_=scr[:],
                                            axis=mybir.AxisListType.X,
                                            op=mybir.AluOpType.add)
                    u = msmallp.tile([128, 1], f32, tag=f"u{st}")
                    nc.vector.tensor_scalar(out=u[:], in0=cost[:],
                                            scalar1=-1.0, scalar2=1.0,
                                            op0=mybir.AluOpType.max,
                                            op1=mybir.AluOpType.min)
                    usq = msmallp.tile([128, 1], f32, tag=f"usq{st}")
                    nc.vector.tensor_tensor(out=usq[:], in0=u[:], in1=u[:],
                                            op=mybir.AluOpType.mult)
                    root = msmallp.tile([128, 1], f32, tag=f"root{st}")
                    nc.scalar.activation(out=root[:], in_=usq[:],
                                         func=mybir.ActivationFunctionType.Sqrt,
                                         scale=-1.0, bias=1.0)
                    parts.append((cost, u, root))

            def _margin_part_b():
                for st in range(NSLOT // 128):
                    cost, u, root = parts[st]
                    t1 = msmallp.tile([128, 1], f32, tag=f"t1{st}")
                    nc.vector.tensor_scalar(out=t1[:], in0=u[:],
                                            scalar1=COS_M, scalar2=None,
                                            op0=mybir.AluOpType.mult)
                    t2 = msmallp.tile([128, 1], f32, tag=f"t2{st}")
                    nc.vector.tensor_scalar(out=t2[:], in0=root[:],
                                            scalar1=SIN_M, scalar2=None,
                                            op0=mybir.AluOpType.mult)
                    newz = msmallp.tile([128, 1], f32, tag=f"newz{st}")
                    nc.vector.tensor_tensor(out=newz[:], in0=t1[:],
                                            in1=t2[:],
                                            op=mybir.AluOpType.subtract)
                    dlt = msmallp.tile([128, 1], f32, tag=f"dlt{st}")
                    nc.vector.tensor_tensor(out=dlt[:], in0=newz[:],
                                            in1=cost[:],
                                            op=mybir.AluOpType.subtract)
                    mask = msmallp.tile([128, 1], f32, tag=f"mask{st}")
                    nc.vector.tensor_scalar(out=mask[:], in0=cost[:],
                                            scalar1=0.0, scalar2=None,
                                            op0=mybir.AluOpType.is_gt)
                    md = msmallp.tile([128, 1], f32, tag=f"md{st}")
                    nc.vector.tensor_tensor(out=md[:], in0=mask[:],
                                            in1=dlt[:],
                                            op=mybir.AluOpType.mult)
                    val = msmallp.tile([128, 1], f32, tag=f"val{st}")
                    nc.vector.tensor_tensor(out=val[:], in0=cost[:],
                                            in1=md[:],
                                            op=mybir.AluOpType.add)
                    corr = msmallp.tile([128, 1], f32, tag=f"corr{st}")
                    nc.vector.tensor_scalar(out=corr[:], in0=val[:],
                                            scalar1=S, scalar2=None,
                                            op0=mybir.AluOpType.mult)
                    nc.gpsimd.dma_start(
                        out=corr_ext[st * 128:(st + 1) * 128, :],
                        in_=corr[:])

            # ---- main matmul stream ---------------------------------------
            first_mm = None
            for g in range(NG):
                for bt in range(BT):
                    ob = obp.tile([128, GCOL], bf16, tag="ob")
                    pos = [psump.tile([128, 512], f32, tag="po",
                                      name=f"po{j}")
                           for j in range(NJ)]
                    # NOTE: keep accumulation groups strictly sequential
                    # (j outer, d inner). Interleaving open groups across
                    # PSUM banks drops the PE into a non-pipelined mode
                    # (~2.4x slower) for the rest of the kernel.
                    for j in range(NJ):
                        for dd in range(DT):
                            mm = nc.tensor.matmul(
                                out=pos[j][:, :JC],
                                lhsT=xnT[dd][:, bt * 128:(bt + 1) * 128],
                                rhs=wT[(g, dd)][:, j * JC:(j + 1) * JC],
                                start=(dd == 0), stop=(dd == DT - 1))
                            if first_mm is None:
                                first_mm = mm
                                for gd in gated_dmas:
                                    add_dep_helper(gd.ins, first_mm.ins,
                                                   sync=True,
                                                   reason="clear prologue dma")
                    first_evac = None
                    for j in range(NJ):
                        osl = slice(j * JC, (j + 1) * JC)
                        if j < 2:
                            ev = nc.vector.tensor_scalar(
                                out=ob[:, osl], in0=pos[j][:, :JC],
                                scalar1=1.0, scalar2=None,
                                op0=mybir.AluOpType.mult)
                            if first_evac is None:
                                first_evac = ev
                        else:
                            nc.scalar.copy(out=ob[:, osl], in_=pos[j][:, :JC])
                    if bt == BT - 1 and g == NG - 1:
                        # split the final store so it starts right after the
                        # first half's evacuations instead of after all four
                        nc.sync.dma_start(
                            out=out_ext[bt * 128:(bt + 1) * 128,
                                        g * GCOL:g * GCOL + 2 * JC],
                            in_=ob[:, :2 * JC])
                        nc.sync.dma_start(
                            out=out_ext[bt * 128:(bt + 1) * 128,
                                        g * GCOL + 2 * JC:(g + 1) * GCOL],
                            in_=ob[:, 2 * JC:])
                    else:
                        nc.sync.dma_start(
                            out=out_ext[bt * 128:(bt + 1) * 128,
                                        g * GCOL:(g + 1) * GCOL],
                            in_=ob[:])
                    if bt == 0 and g + 2 < NG:
                        # stagger the weight prefetch for group g+2: issue
                        # only after group g has started evacuating
                        for d in range(DT):
                            k = (g + 2) * DT + d
                            dma = nc.gpsimd.dma_start(
                                out=wT[(g + 2, d)][:],
                                in_=wtr_ext[k * 128:(k + 1) * 128, :])
                            add_dep_helper(dma.ins, first_evac.ins, sync=True,
                                           reason="stagger wt prefetch")
                    if g == 0 and bt == 4:
                        _margin_part_a()
                    if g == 1 and bt == 1:
                        _margin_part_b()

    nc.finalize()
    return nc


def _get_graph():
    if "nc" not in _CACHE:
        _CACHE["nc"] = _build_graph()
    return _CACHE["nc"]


def _margin_val(cos_t):
    """Reference margin math (f32 scalar), for host fallback slots."""
    u = min(max(float(cos_t), -1.0), 1.0)
    new = COS_M * u - SIN_M * math.sqrt(max(0.0, 1.0 - u * u))
    return S * (new if cos_t > 0 else float(cos_t))


def kernel(x, weight, target):
    x = np.ascontiguousarray(np.asarray(x, dtype=np.float32))
    weight = np.ascontiguousarray(np.asarray(weight, dtype=np.float32))
    target = np.asarray(target).astype(np.int64)

    nc = _get_graph()

    # host-side l2 normalization (f32, matching the reference's eps clamp)
    xn = x / np.maximum(
        np.sqrt(np.einsum("bd,bd->b", x, x)[:, None]), EPS).astype(np.float32)
    wnrm = np.sqrt(np.einsum("cd,cd->c", weight, weight))[:, None]
    wn = weight / np.maximum(wnrm, EPS).astype(np.float32)

    xt = np.ascontiguousarray((S * xn).T).astype(ml_dtypes.bfloat16)

    e1 = np.zeros((D,), dtype=np.float32)
    e1[0] = 1.0

    in_maps = []
    slot_rows = []      # per core: row indices whose corr the device computes
    host_fixups = []    # (b, t, value) computed on host for overflow slots
    for c in range(NCORES):
        c0 = c * CS
        wsh = np.zeros((D, CSP), dtype=ml_dtypes.bfloat16)
        wsh[:, :CS] = wn[c0:c0 + CS].T.astype(ml_dtypes.bfloat16)
        # pack [128,1792] tiles contiguously, (g,d)-major
        wtr = np.empty((NG * DT * 128, GCOL), dtype=ml_dtypes.bfloat16)
        for g in range(NG):
            for d in range(DT):
                k = g * DT + d
                wtr[k * 128:(k + 1) * 128] = \
                    wsh[d * 128:(d + 1) * 128, g * GCOL:(g + 1) * GCOL]

        rows = np.nonzero((target >= c0) & (target < c0 + CS))[0]
        dev_rows = rows[:NSLOT]
        for b in rows[NSLOT:]:
            t = int(target[b])
            cos_t = float(xn[b] @ wn[t])
            host_fixups.append((int(b), t, _margin_val(cos_t)))
        xs = np.broadcast_to(e1, (NSLOT, D)).copy()
        ws = np.broadcast_to(e1, (NSLOT, D)).copy()
        n = len(dev_rows)
        xs[:n] = xn[dev_rows]
        ws[:n] = wn[target[dev_rows]]
        xs = xs.astype(ml_dtypes.bfloat16)
        ws = ws.astype(ml_dtypes.bfloat16)
        slot_rows.append(dev_rows)
        in_maps.append({"xt": xt, "wtr": wtr, "xs": xs, "ws": ws})

    from concourse.bass_utils import run_bass_kernel_spmd
    res = None
    last_err = None
    for attempt in range(3):
        try:
            res = run_bass_kernel_spmd(nc, in_maps, core_ids=list(range(NCORES)))
            break
        except Exception as e:  # transient NRT_EXEC_UNIT_UNRECOVERABLE flakes
            last_err = e
            import time as _time
            _time.sleep(5)
    if res is None:
        raise last_err

    out = np.concatenate(
        [res.results[c]["out"][:, :CS].astype(np.float32) for c in range(NCORES)],
        axis=1)

    # place the device-computed margin values during unshard
    for c in range(NCORES):
        rows = slot_rows[c]
        if len(rows):
            corr = res.results[c]["corr"][:len(rows), 0].astype(np.float32)
            out[rows, target[rows]] = corr
    for b, t, v in host_fixups:
        out[b, t] = v
    return out
